# revision 24
# baseline (speedup 1.0000x reference)
"""Biaffine span classifier kernel for 8 Trainium2 NeuronCores.

Math (per batch b, label o):
    start = relu(x @ W_start + b_start); end = relu(x @ W_end + b_end)
    rotate both with tiled-halves sinusoidal tables
    span[o,x,y] = startR[x,:] @ weight[o] @ endR[y,:]^T
    span = span*pad[y] - (1-pad[y])*NEG - NEG*tril(x>y)

Sharding: core c = b*2 + half handles batch b and labels [half*8, half*8+8).
Each core writes a contiguous [8, S, S] slice of the output.

On-chip layout is transposed ([H, S], H on partitions); x is transposed on
the host so every contraction has its reduction dim on partitions. All
matmuls run in fp32r mode (single-pass PE, ~TF32 precision) — operands are
rounded to fp32r by their producers as the BIR verifier requires. Start and
end projections share one matmul chain (stacked [W_start|W_end] stationary
operand); the end half is moved to partitions 0-63 by selector matmuls that
also produce the rotation's pair-swapped values. The mask's additive term is
folded into the big matmul via an augmented K=65 contraction (ones row in
tmpT, add_row in endT). Blocks entirely below the diagonal are exactly -NEG
in fp32 (|span| << 0.5*ulp(NEG)); their output regions are written once
during prep from a constant band on the SWDGE queue, hiding that DMA under
setup compute. Each label's remaining output goes out as a 2 MB contiguous
chunk (rows 0-511) plus a 1 MB strided chunk (rows 512-1023, y >= 512),
double-buffered so DMA, PE, DVE and ACT overlap.
"""

import numpy as np

B, S, I, H, O = 4, 1024, 768, 64, 16
NCORES = 8
OH = O // 2  # 8 labels per core
NEG = 1.0e12
KT = I // 128  # 6 k-tiles over the input dim
ST = S // 128  # 8 s-tiles

_STATE = {}


def _tables():
    """Host-precomputed constants (mimic reference fp32 ops)."""
    position = np.arange(S, dtype=np.float32)
    idx = np.arange(H // 2, dtype=np.float32)
    expo = (np.float32(-2.0) * idx) / np.float32(H)
    inv_freq = np.power(np.float32(10000.0), expo).astype(np.float32)
    ang = position[:, None] * inv_freq[None, :]          # [S, 32] f32
    cos_h = np.cos(ang).astype(np.float32).T             # [32, S]
    sin_h = np.sin(ang).astype(np.float32).T
    cosT = np.ascontiguousarray(np.concatenate([cos_h, cos_h], axis=0))  # [64, S]
    sinT = np.ascontiguousarray(np.concatenate([sin_h, sin_h], axis=0))
    # pair-swap as lhsT: out[2m] = -in[2m+1]; out[2m+1] = in[2m]
    msw = np.zeros((H, H), np.float32)
    for m in range(H // 2):
        msw[2 * m + 1, 2 * m] = -1.0
        msw[2 * m, 2 * m + 1] = 1.0
    # selectors on the stacked [start; end] projection (lhsT, [128, 192]):
    # [:, 0:64] swap start rows; [:, 64:128] extract end rows; [:, 128:192]
    # swap end rows
    sel = np.zeros((2 * H, 3 * H), np.float32)
    sel[0:H, 0:H] = msw
    sel[H:2 * H, H:2 * H] = np.eye(H, dtype=np.float32)
    sel[H:2 * H, 2 * H:3 * H] = msw
    # compressed tril window: T[x', v] = -NEG if x'+384 > v; pattern k for
    # diagonal-crossing blocks is the slice [:, 384-128k : 896-128k]
    xp = np.arange(128, dtype=np.int64)[:, None]
    vp = np.arange(896, dtype=np.int64)[None, :]
    tril = np.where(xp + 384 > vp, np.float32(-NEG),
                    np.float32(0.0)).astype(np.float32)   # [128, 896]
    return cosT, sinT, sel, tril


def _build():
    import concourse.bacc as bacc
    import concourse.bass as bass
    import concourse.mybir as mybir
    from concourse import tile

    f32 = mybir.dt.float32
    f32r = mybir.dt.float32r
    AF = mybir.ActivationFunctionType
    ALU = mybir.AluOpType
    PSUM = bass.MemorySpace.PSUM

    nc = bacc.Bacc("TRN2", target_bir_lowering=False, debug=False,
                   num_devices=NCORES)

    xT_t = nc.dram_tensor("xT", [I, S], f32, kind="ExternalInput")
    mask_t = nc.dram_tensor("mask", [1, S], f32, kind="ExternalInput")
    wb_t = nc.dram_tensor("w_both", [I, 2 * H], f32, kind="ExternalInput")
    b2_t = nc.dram_tensor("bias2", [2 * H, 1], f32, kind="ExternalInput")
    wo_t = nc.dram_tensor("w_o", [OH, H, H], f32, kind="ExternalInput")
    cos_t = nc.dram_tensor("cos_t", [H, S], f32, kind="ExternalInput")
    sin_t = nc.dram_tensor("sin_t", [H, S], f32, kind="ExternalInput")
    sel_t = nc.dram_tensor("sel3", [2 * H, 3 * H], f32, kind="ExternalInput")
    tril_t = nc.dram_tensor("trilneg", [128, 896], f32, kind="ExternalInput")
    out_t = nc.dram_tensor("out", [OH, S, S], f32, kind="ExternalOutput")

    # [o, c, p, xb, y]: row = 512c + 128xb + p
    out_r = out_t.ap().rearrange("o (c xb p) y -> o c p xb y", c=2, xb=4, p=128)

    def r(ap):
        return ap.bitcast(f32r)

    with tile.TileContext(nc) as tc:
        with tc.tile_pool(name="persist", bufs=1) as pp, \
             tc.tile_pool(name="scratch", bufs=2) as sp:
            wbT = pp.tile([128, KT, 2 * H], f32)
            sel3 = pp.tile([2 * H, 3 * H], f32)
            wo = pp.tile([H, OH, H], f32)
            xTr = pp.tile([128, KT, S], f32)
            mask0r = pp.tile([1, S], f32)
            bias2 = pp.tile([2 * H, 1], f32)
            cosT = pp.tile([H, S], f32)
            sinT = pp.tile([H, S], f32)
            tril = pp.tile([128, 896], f32)
            startR = pp.tile([H, S], f32)
            endA = pp.tile([H + 1, S], f32)       # 0..63 endR*pad, 64 addrow
            padB = pp.tile([H, S], f32)
            constband = pp.tile([128, 4, 512], f32)  # 4 copies of const band
            addrow0 = pp.tile([1, S], f32)
            mask0 = pp.tile([1, S], f32)

            with tc.tile_pool(name="load", bufs=1) as lp:
                # mask first: it gates the constant-band writes, which should
                # saturate DMA while the rest of prep computes
                nc.sync.dma_start(mask0[:], mask_t.ap())
                ones1f = pp.tile([1, 128], f32)
                nc.gpsimd.memset(ones1f[:], 1.0)
                ones1 = pp.tile([1, 128], f32)
                nc.vector.tensor_copy(r(ones1[:]), ones1f[:])
                onesrow = pp.tile([1, S], f32)
                nc.gpsimd.memset(onesrow[:], 1.0)
                onesrowr = pp.tile([1, S], f32)
                nc.vector.tensor_copy(r(onesrowr[:]), onesrow[:])
                nc.vector.tensor_copy(r(mask0r[:]), mask0[:])
                nc.vector.tensor_scalar(
                    r(addrow0[:]), mask0[:], float(NEG), float(-NEG),
                    ALU.mult, ALU.add)                 # (pad-1)*NEG

                # critical-path loads: projection weights + xT chunks
                wbL = lp.tile([128, KT, 2 * H], f32)
                nc.sync.dma_start(
                    wbL[:], wb_t.ap().rearrange("(t p) h -> p t h", p=128))
                nc.vector.tensor_copy(r(wbT[:]), wbL[:])
                selL = lp.tile([2 * H, 3 * H], f32)
                nc.sync.dma_start(selL[:], sel_t.ap())
                nc.scalar.copy(r(sel3[:]), selL[:])
                xTin = lp.tile([128, KT, S], f32)
                xg = xT_t.ap().rearrange("(t p) s -> p t s", p=128)
                for t in range(KT):
                    nc.sync.dma_start(xTin[:, t, :], xg[:, t, :])
                    if t % 2 == 0:
                        nc.vector.tensor_copy(r(xTr[:, t, :]), xTin[:, t, :])
                    else:
                        nc.scalar.copy(r(xTr[:, t, :]), xTin[:, t, :])
                woL = lp.tile([H, OH, H], f32)
                nc.sync.dma_start(woL[:], wo_t.ap().rearrange("o i j -> i o j"))
                nc.scalar.copy(r(wo[:]), woL[:])
                nc.sync.dma_start(bias2[:], b2_t.ap())
                nc.sync.dma_start(cosT[:], cos_t.ap())
                nc.sync.dma_start(sinT[:], sin_t.ap())
                nc.sync.dma_start(tril[:], tril_t.ap())

                with tc.tile_pool(name="ps_proj", bufs=2, space=PSUM) as ppj:
                    # pad broadcast + constant band via K=1 fp32r matmuls
                    for h in range(2):
                        sl = slice(h * 512, (h + 1) * 512)
                        ps_pb = ppj.tile([H, 512], f32, name="ps_pb", bufs=1)
                        nc.tensor.matmul(ps_pb[:], r(ones1[:, :H]),
                                         r(mask0r[:, sl]),
                                         start=True, stop=True)
                        nc.scalar.copy(padB[:, sl], ps_pb[:])
                    ps_cb = ppj.tile([128, 512], f32, name="ps_cb", bufs=1)
                    nc.tensor.matmul(ps_cb[:], r(ones1[:]), r(addrow0[:, 0:512]),
                                     start=True, stop=True)
                    nc.scalar.activation(constband[:, 0, :], ps_cb[:], AF.Copy,
                                         bias=float(-NEG))
                    for j in range(1, 4):
                        nc.scalar.copy(constband[:, j, :], constband[:, 0, :])
                    # constant (below-diagonal) output regions for every label:
                    # rows 512..1023, y < 512 — write them now, while DMA is
                    # otherwise idle, on the SWDGE queue so they can't block
                    # the small HWDGE moves that gate the first span matmuls.
                    for o in range(OH):
                        nc.gpsimd.dma_start(out_r[o, 1][:, :, 0:512],
                                            constband[:])

                    # ---- fused projections + rotation ----
                    for h in range(2):
                        sl = slice(h * 512, (h + 1) * 512)
                        ps2 = ppj.tile([128, 512], f32, name="ps2")
                        for kb in range(KT):
                            nc.tensor.matmul(
                                ps2[:], r(wbT[:, kb, :]), r(xTr[:, kb, sl]),
                                start=(kb == 0), stop=(kb == KT - 1))
                        relu2 = sp.tile([128, 512], f32, name="relu2")
                        nc.scalar.activation(r(relu2[:]), ps2[:], AF.Relu,
                                             bias=bias2[:])
                        swS = ppj.tile([H, 512], f32, name="swS", tag="aux",
                                       bufs=3)
                        nc.tensor.matmul(swS[:], r(sel3[:, 0:H]), r(relu2[:]),
                                         start=True, stop=True)
                        exE = ppj.tile([H, 512], f32, name="exE", tag="aux",
                                       bufs=3)
                        nc.tensor.matmul(exE[:], r(sel3[:, H:2 * H]),
                                         r(relu2[:]), start=True, stop=True)
                        swE = ppj.tile([H, 512], f32, name="swE", tag="aux",
                                       bufs=3)
                        nc.tensor.matmul(swE[:], r(sel3[:, 2 * H:3 * H]),
                                         r(relu2[:]), start=True, stop=True)
                        # start side: rows 0-63 of relu2 are unswapped start
                        rm = sp.tile([H, 512], f32, name="rm")
                        nc.vector.tensor_mul(rm[:], relu2[0:H, :], cosT[:, sl])
                        rs = sp.tile([H, 512], f32, name="rs")
                        nc.vector.tensor_mul(rs[:], swS[:], sinT[:, sl])
                        nc.vector.tensor_add(r(startR[:, sl]), rm[:], rs[:])
                        # end side
                        rm2 = sp.tile([H, 512], f32, name="rm2")
                        nc.vector.tensor_mul(rm2[:], exE[:], cosT[:, sl])
                        rs2 = sp.tile([H, 512], f32, name="rs2")
                        nc.vector.tensor_mul(rs2[:], swE[:], sinT[:, sl])
                        es = sp.tile([H, 512], f32, name="es")
                        nc.vector.tensor_add(es[:], rm2[:], rs2[:])
                        nc.vector.tensor_mul(r(endA[0:H, sl]), es[:],
                                             padB[:, sl])
                # row 64 of endA = addrow (cross-partition move via DMA; scalar
                # HWDGE ring so the big sync-queue transfers can't delay it)
                nc.scalar.dma_start(r(endA[H:H + 1, :]), r(addrow0[:]))

            # ---- label double-buffers ----
            tmpA0 = pp.tile([H + 1, S], f32)
            tmpA1 = pp.tile([H + 1, S], f32)
            nc.scalar.dma_start(r(tmpA0[H:H + 1, :]), r(onesrowr[:]))
            nc.scalar.dma_start(r(tmpA1[H:H + 1, :]), r(onesrowr[:]))

            # ---- main loop over labels ----
            def trilpat(k):
                return tril[:, 384 - 128 * k:896 - 128 * k]

            with tc.tile_pool(name="stg0_pool", bufs=3) as st0, \
                 tc.tile_pool(name="stg1_pool", bufs=3) as st1, \
                 tc.tile_pool(name="ps_main", bufs=2, space=PSUM) as pm, \
                 tc.tile_pool(name="ps_span_pool", bufs=6, space=PSUM) as pspan:
                for o in range(OH):
                    tmpA = tmpA0 if o % 2 == 0 else tmpA1
                    # tmpT[j, x] = sum_i weight[o][i, j] * startR[i, x]
                    for h in range(2):
                        sl = slice(h * 512, (h + 1) * 512)
                        ps_tmp = pm.tile([H, 512], f32, name="ps_tmp")
                        nc.tensor.matmul(ps_tmp[:],
                                         r(wo[:, o, :]), r(startR[:, sl]),
                                         start=True, stop=True)
                        nc.scalar.copy(r(tmpA[0:H, sl]), ps_tmp[:])
                    # chunk 1 first (smaller, 4 matmuls): rows 512-1023,
                    # computed y half only
                    stg1 = st1.tile([128, 4, 512], f32, name="stg1")
                    for xb in range(4, 8):
                        lhs = r(tmpA[:, xb * 128:(xb + 1) * 128])
                        ps_sp3 = pspan.tile([128, 512], f32, name="ps_sp3",
                                            tag="ps_sp")
                        nc.tensor.matmul(ps_sp3[:], lhs, r(endA[:, 512:1024]),
                                         start=True, stop=True)
                        nc.vector.tensor_tensor(stg1[:, xb - 4, :], ps_sp3[:],
                                                trilpat(xb - 4), ALU.add)
                    nc.sync.dma_start(out_r[o, 1][:, :, 512:1024], stg1[:])
                    # chunk 0: rows 0-511 (xb 0-3), both y halves
                    stg0 = st0.tile([128, 4, S], f32, name="stg0")
                    for xb in range(4):
                        lhs = r(tmpA[:, xb * 128:(xb + 1) * 128])
                        ps_sp = pspan.tile([128, 512], f32, name="ps_sp",
                                           tag="ps_sp")
                        nc.tensor.matmul(ps_sp[:], lhs, r(endA[:, 0:512]),
                                         start=True, stop=True)
                        nc.vector.tensor_tensor(stg0[:, xb, 0:512], ps_sp[:],
                                                trilpat(xb), ALU.add)
                        ps_sp2 = pspan.tile([128, 512], f32, name="ps_sp2",
                                            tag="ps_sp")
                        nc.tensor.matmul(ps_sp2[:], lhs, r(endA[:, 512:1024]),
                                         start=True, stop=True)
                        nc.scalar.copy(stg0[:, xb, 512:1024], ps_sp2[:])
                    nc.sync.dma_start(out_r[o, 0], stg0[:])

    nc.compile()
    return nc


def _get_nc():
    if "nc" not in _STATE:
        _STATE["nc"] = _build()
    return _STATE["nc"]


def _make_in_maps(x, mask, W_start, b_start, W_end, b_end, weight):
    cosT, sinT, sel, tril = _tables()
    x = np.asarray(x, np.float32)
    mask = np.ascontiguousarray(np.asarray(mask, np.float32))
    W_start = np.asarray(W_start, np.float32)
    W_end = np.asarray(W_end, np.float32)
    w_both = np.ascontiguousarray(np.concatenate([W_start, W_end], axis=1))
    bias2 = np.ascontiguousarray(
        np.concatenate([np.asarray(b_start, np.float32).reshape(H),
                        np.asarray(b_end, np.float32).reshape(H)]).reshape(
                            2 * H, 1))
    weight = np.ascontiguousarray(np.asarray(weight, np.float32))
    in_maps = []
    for c in range(NCORES):
        b, half = c // 2, c % 2
        in_maps.append({
            "xT": np.ascontiguousarray(x[b].T),
            "mask": np.ascontiguousarray(mask[b:b + 1]),
            "w_both": w_both,
            "bias2": bias2,
            "w_o": np.ascontiguousarray(weight[half * OH:(half + 1) * OH]),
            "cos_t": cosT,
            "sin_t": sinT,
            "sel3": sel,
            "trilneg": tril,
        })
    return in_maps


def _execute(in_maps, trace=False):
    from concourse.bass_utils import run_bass_kernel_spmd
    nc = _get_nc()
    return run_bass_kernel_spmd(nc, in_maps, list(range(NCORES)), trace=trace)


def kernel(x, mask, W_start, b_start, W_end, b_end, weight):
    in_maps = _make_in_maps(x, mask, W_start, b_start, W_end, b_end, weight)
    res = _execute(in_maps)
    outs = [res.results[c]["out"] for c in range(NCORES)]
    full = np.stack(outs).reshape(B, 2, OH, S, S).reshape(B, O, S, S)
    return full.astype(np.float32)


# revision 25
# speedup vs baseline: 1.0023x; 1.0023x over previous
"""Biaffine span classifier kernel for 8 Trainium2 NeuronCores.

Math (per batch b, label o):
    start = relu(x @ W_start + b_start); end = relu(x @ W_end + b_end)
    rotate both with tiled-halves sinusoidal tables
    span[o,x,y] = startR[x,:] @ weight[o] @ endR[y,:]^T
    span = span*pad[y] - (1-pad[y])*NEG - NEG*tril(x>y)

Sharding: core c = b*2 + half handles batch b and labels [half*8, half*8+8).
Each core writes a contiguous [8, S, S] slice of the output.

On-chip layout is transposed ([H, S], H on partitions); x is transposed on
the host so every contraction has its reduction dim on partitions. All
matmuls run in fp32r mode (single-pass PE, ~TF32 precision) — operands are
rounded to fp32r by their producers as the BIR verifier requires. Start and
end projections share one matmul chain (stacked [W_start|W_end] stationary
operand); the end half is moved to partitions 0-63 by selector matmuls that
also produce the rotation's pair-swapped values. The mask's additive term is
folded into the big matmul via an augmented K=65 contraction (ones row in
tmpT, add_row in endT). Blocks entirely below the diagonal are exactly -NEG
in fp32 (|span| << 0.5*ulp(NEG)); their output regions are written once
during prep from a constant band on the SWDGE queue, hiding that DMA under
setup compute. Each label's remaining output goes out as a 2 MB contiguous
chunk (rows 0-511) plus a 1 MB strided chunk (rows 512-1023, y >= 512),
double-buffered so DMA, PE, DVE and ACT overlap.
"""

import numpy as np

B, S, I, H, O = 4, 1024, 768, 64, 16
NCORES = 8
OH = O // 2  # 8 labels per core
NEG = 1.0e12
KT = I // 128  # 6 k-tiles over the input dim
ST = S // 128  # 8 s-tiles

_STATE = {}


def _tables():
    """Host-precomputed constants (mimic reference fp32 ops)."""
    position = np.arange(S, dtype=np.float32)
    idx = np.arange(H // 2, dtype=np.float32)
    expo = (np.float32(-2.0) * idx) / np.float32(H)
    inv_freq = np.power(np.float32(10000.0), expo).astype(np.float32)
    ang = position[:, None] * inv_freq[None, :]          # [S, 32] f32
    cos_h = np.cos(ang).astype(np.float32).T             # [32, S]
    sin_h = np.sin(ang).astype(np.float32).T
    cosT = np.ascontiguousarray(np.concatenate([cos_h, cos_h], axis=0))  # [64, S]
    sinT = np.ascontiguousarray(np.concatenate([sin_h, sin_h], axis=0))
    # pair-swap as lhsT: out[2m] = -in[2m+1]; out[2m+1] = in[2m]
    msw = np.zeros((H, H), np.float32)
    for m in range(H // 2):
        msw[2 * m + 1, 2 * m] = -1.0
        msw[2 * m, 2 * m + 1] = 1.0
    # selectors on the stacked [start; end] projection (lhsT, [128, 192]):
    # [:, 0:64] swap start rows; [:, 64:128] extract end rows; [:, 128:192]
    # swap end rows
    sel = np.zeros((2 * H, 3 * H), np.float32)
    sel[0:H, 0:H] = msw
    sel[H:2 * H, H:2 * H] = np.eye(H, dtype=np.float32)
    sel[H:2 * H, 2 * H:3 * H] = msw
    # compressed tril window: T[x', v] = -NEG if x'+384 > v; pattern k for
    # diagonal-crossing blocks is the slice [:, 384-128k : 896-128k]
    xp = np.arange(128, dtype=np.int64)[:, None]
    vp = np.arange(896, dtype=np.int64)[None, :]
    tril = np.where(xp + 384 > vp, np.float32(-NEG),
                    np.float32(0.0)).astype(np.float32)   # [128, 896]
    return cosT, sinT, sel, tril


def _build():
    import concourse.bacc as bacc
    import concourse.bass as bass
    import concourse.mybir as mybir
    from concourse import tile

    f32 = mybir.dt.float32
    f32r = mybir.dt.float32r
    AF = mybir.ActivationFunctionType
    ALU = mybir.AluOpType
    PSUM = bass.MemorySpace.PSUM

    nc = bacc.Bacc("TRN2", target_bir_lowering=False, debug=False,
                   num_devices=NCORES)

    xT_t = nc.dram_tensor("xT", [I, S], f32, kind="ExternalInput")
    mask_t = nc.dram_tensor("mask", [1, S], f32, kind="ExternalInput")
    wb_t = nc.dram_tensor("w_both", [I, 2 * H], f32, kind="ExternalInput")
    b2_t = nc.dram_tensor("bias2", [2 * H, 1], f32, kind="ExternalInput")
    wo_t = nc.dram_tensor("w_o", [OH, H, H], f32, kind="ExternalInput")
    cos_t = nc.dram_tensor("cos_t", [H, S], f32, kind="ExternalInput")
    sin_t = nc.dram_tensor("sin_t", [H, S], f32, kind="ExternalInput")
    sel_t = nc.dram_tensor("sel3", [2 * H, 3 * H], f32, kind="ExternalInput")
    tril_t = nc.dram_tensor("trilneg", [128, 896], f32, kind="ExternalInput")
    out_t = nc.dram_tensor("out", [OH, S, S], f32, kind="ExternalOutput")

    # [o, c, p, xb, y]: row = 512c + 128xb + p
    out_r = out_t.ap().rearrange("o (c xb p) y -> o c p xb y", c=2, xb=4, p=128)

    def r(ap):
        return ap.bitcast(f32r)

    with tile.TileContext(nc) as tc:
        with tc.tile_pool(name="persist", bufs=1) as pp, \
             tc.tile_pool(name="scratch", bufs=2) as sp:
            wbT = pp.tile([128, KT, 2 * H], f32)
            sel3 = pp.tile([2 * H, 3 * H], f32)
            wo = pp.tile([H, OH, H], f32)
            xTr = pp.tile([128, KT, S], f32)
            mask0r = pp.tile([1, S], f32)
            bias2 = pp.tile([2 * H, 1], f32)
            cosT = pp.tile([H, S], f32)
            sinT = pp.tile([H, S], f32)
            tril = pp.tile([128, 896], f32)
            startR = pp.tile([H, S], f32)
            endA = pp.tile([H + 1, S], f32)       # 0..63 endR*pad, 64 addrow
            padB = pp.tile([H, S], f32)
            constband = pp.tile([128, 4, 512], f32)  # 4 copies of const band
            addrow0 = pp.tile([1, S], f32)
            mask0 = pp.tile([1, S], f32)

            with tc.tile_pool(name="load", bufs=1) as lp:
                # mask first: it gates the constant-band writes, which should
                # saturate DMA while the rest of prep computes
                nc.sync.dma_start(mask0[:], mask_t.ap())
                ones1f = pp.tile([1, 128], f32)
                nc.gpsimd.memset(ones1f[:], 1.0)
                ones1 = pp.tile([1, 128], f32)
                nc.vector.tensor_copy(r(ones1[:]), ones1f[:])
                onesrow = pp.tile([1, S], f32)
                nc.gpsimd.memset(onesrow[:], 1.0)
                onesrowr = pp.tile([1, S], f32)
                nc.vector.tensor_copy(r(onesrowr[:]), onesrow[:])
                nc.vector.tensor_copy(r(mask0r[:]), mask0[:])
                nc.vector.tensor_scalar(
                    r(addrow0[:]), mask0[:], float(NEG), float(-NEG),
                    ALU.mult, ALU.add)                 # (pad-1)*NEG

                # critical-path loads: projection weights + xT chunks
                wbL = lp.tile([128, KT, 2 * H], f32)
                nc.sync.dma_start(
                    wbL[:], wb_t.ap().rearrange("(t p) h -> p t h", p=128))
                nc.vector.tensor_copy(r(wbT[:]), wbL[:])
                selL = lp.tile([2 * H, 3 * H], f32)
                nc.sync.dma_start(selL[:], sel_t.ap())
                nc.scalar.copy(r(sel3[:]), selL[:])
                xTin = lp.tile([128, KT, S], f32)
                xg = xT_t.ap().rearrange("(t p) s -> p t s", p=128)
                for t in range(KT):
                    nc.sync.dma_start(xTin[:, t, :], xg[:, t, :])
                    if t % 2 == 0:
                        nc.vector.tensor_copy(r(xTr[:, t, :]), xTin[:, t, :])
                    else:
                        nc.scalar.copy(r(xTr[:, t, :]), xTin[:, t, :])
                woL = lp.tile([H, OH, H], f32)
                nc.sync.dma_start(woL[:], wo_t.ap().rearrange("o i j -> i o j"))
                nc.scalar.copy(r(wo[:]), woL[:])
                nc.sync.dma_start(bias2[:], b2_t.ap())
                nc.sync.dma_start(cosT[:], cos_t.ap())
                nc.sync.dma_start(sinT[:], sin_t.ap())
                nc.sync.dma_start(tril[:], tril_t.ap())

                with tc.tile_pool(name="ps_proj", bufs=2, space=PSUM) as ppj:
                    # pad broadcast + constant band via K=1 fp32r matmuls
                    for h in range(2):
                        sl = slice(h * 512, (h + 1) * 512)
                        ps_pb = ppj.tile([H, 512], f32, name="ps_pb", bufs=1)
                        nc.tensor.matmul(ps_pb[:], r(ones1[:, :H]),
                                         r(mask0r[:, sl]),
                                         start=True, stop=True)
                        nc.scalar.copy(padB[:, sl], ps_pb[:])
                    ps_cb = ppj.tile([128, 512], f32, name="ps_cb", bufs=1)
                    nc.tensor.matmul(ps_cb[:], r(ones1[:]), r(addrow0[:, 0:512]),
                                     start=True, stop=True)
                    nc.scalar.activation(constband[:, 0, :], ps_cb[:], AF.Copy,
                                         bias=float(-NEG))
                    for j in range(1, 4):
                        nc.scalar.copy(constband[:, j, :], constband[:, 0, :])
                    # constant (below-diagonal) output regions for every label:
                    # rows 512..1023, y < 512 — write them now, while DMA is
                    # otherwise idle, on the SWDGE queue so they can't block
                    # the small HWDGE moves that gate the first span matmuls.
                    for o in range(OH):
                        nc.gpsimd.dma_start(out_r[o, 1][:, :, 0:512],
                                            constband[:])

                    # ---- fused projections + rotation ----
                    for h in range(2):
                        sl = slice(h * 512, (h + 1) * 512)
                        ps2 = ppj.tile([128, 512], f32, name="ps2")
                        for kb in range(KT):
                            nc.tensor.matmul(
                                ps2[:], r(wbT[:, kb, :]), r(xTr[:, kb, sl]),
                                start=(kb == 0), stop=(kb == KT - 1))
                        relu2 = sp.tile([128, 512], f32, name="relu2")
                        nc.scalar.activation(r(relu2[:]), ps2[:], AF.Relu,
                                             bias=bias2[:])
                        swS = ppj.tile([H, 512], f32, name="swS", tag="aux",
                                       bufs=3)
                        nc.tensor.matmul(swS[:], r(sel3[:, 0:H]), r(relu2[:]),
                                         start=True, stop=True)
                        exE = ppj.tile([H, 512], f32, name="exE", tag="aux",
                                       bufs=3)
                        nc.tensor.matmul(exE[:], r(sel3[:, H:2 * H]),
                                         r(relu2[:]), start=True, stop=True)
                        swE = ppj.tile([H, 512], f32, name="swE", tag="aux",
                                       bufs=3)
                        nc.tensor.matmul(swE[:], r(sel3[:, 2 * H:3 * H]),
                                         r(relu2[:]), start=True, stop=True)
                        # start side: rows 0-63 of relu2 are unswapped start
                        rm = sp.tile([H, 512], f32, name="rm")
                        nc.vector.tensor_mul(rm[:], relu2[0:H, :], cosT[:, sl])
                        rs = sp.tile([H, 512], f32, name="rs")
                        nc.vector.tensor_mul(rs[:], swS[:], sinT[:, sl])
                        nc.vector.tensor_add(r(startR[:, sl]), rm[:], rs[:])
                        # end side
                        rm2 = sp.tile([H, 512], f32, name="rm2")
                        nc.vector.tensor_mul(rm2[:], exE[:], cosT[:, sl])
                        rs2 = sp.tile([H, 512], f32, name="rs2")
                        nc.vector.tensor_mul(rs2[:], swE[:], sinT[:, sl])
                        es = sp.tile([H, 512], f32, name="es")
                        nc.vector.tensor_add(es[:], rm2[:], rs2[:])
                        nc.vector.tensor_mul(r(endA[0:H, sl]), es[:],
                                             padB[:, sl])
                # row 64 of endA = addrow (cross-partition move via DMA; scalar
                # HWDGE ring so the big sync-queue transfers can't delay it)
                nc.scalar.dma_start(r(endA[H:H + 1, :]), r(addrow0[:]))

            # ---- label double-buffers ----
            tmpA0 = pp.tile([H + 1, S], f32)
            tmpA1 = pp.tile([H + 1, S], f32)
            nc.scalar.dma_start(r(tmpA0[H:H + 1, :]), r(onesrowr[:]))
            nc.scalar.dma_start(r(tmpA1[H:H + 1, :]), r(onesrowr[:]))

            # ---- main loop over labels ----
            def trilpat(k):
                return tril[:, 384 - 128 * k:896 - 128 * k]

            with tc.tile_pool(name="stg0_pool", bufs=3) as st0, \
                 tc.tile_pool(name="stg1_pool", bufs=3) as st1, \
                 tc.tile_pool(name="ps_main", bufs=2, space=PSUM) as pm, \
                 tc.tile_pool(name="ps_span_pool", bufs=6, space=PSUM) as pspan:
                for o in range(OH):
                    tmpA = tmpA0 if o % 2 == 0 else tmpA1
                    # tmpT[j, x] = sum_i weight[o][i, j] * startR[i, x]
                    for h in range(2):
                        sl = slice(h * 512, (h + 1) * 512)
                        ps_tmp = pm.tile([H, 512], f32, name="ps_tmp")
                        nc.tensor.matmul(ps_tmp[:],
                                         r(wo[:, o, :]), r(startR[:, sl]),
                                         start=True, stop=True)
                        nc.scalar.copy(r(tmpA[0:H, sl]), ps_tmp[:])
                    # chunk 1 first (smaller, 4 matmuls): rows 512-1023,
                    # computed y half only
                    stg1 = st1.tile([128, 4, 512], f32, name="stg1")
                    for xb in range(4, 8):
                        lhs = r(tmpA[:, xb * 128:(xb + 1) * 128])
                        ps_sp3 = pspan.tile([128, 512], f32, name="ps_sp3",
                                            tag="ps_sp")
                        nc.tensor.matmul(ps_sp3[:], lhs, r(endA[:, 512:1024]),
                                         start=True, stop=True)
                        nc.vector.tensor_tensor(stg1[:, xb - 4, :], ps_sp3[:],
                                                trilpat(xb - 4), ALU.add)
                    # SWDGE ring: gpsimd is idle in the main loop, and a second
                    # ring keeps chunk1 from queuing behind chunk0 transfers
                    nc.gpsimd.dma_start(out_r[o, 1][:, :, 512:1024], stg1[:])
                    # chunk 0: rows 0-511 (xb 0-3), both y halves
                    stg0 = st0.tile([128, 4, S], f32, name="stg0")
                    for xb in range(4):
                        lhs = r(tmpA[:, xb * 128:(xb + 1) * 128])
                        ps_sp = pspan.tile([128, 512], f32, name="ps_sp",
                                           tag="ps_sp")
                        nc.tensor.matmul(ps_sp[:], lhs, r(endA[:, 0:512]),
                                         start=True, stop=True)
                        nc.vector.tensor_tensor(stg0[:, xb, 0:512], ps_sp[:],
                                                trilpat(xb), ALU.add)
                        ps_sp2 = pspan.tile([128, 512], f32, name="ps_sp2",
                                            tag="ps_sp")
                        nc.tensor.matmul(ps_sp2[:], lhs, r(endA[:, 512:1024]),
                                         start=True, stop=True)
                        nc.scalar.copy(stg0[:, xb, 512:1024], ps_sp2[:])
                    nc.sync.dma_start(out_r[o, 0], stg0[:])

    nc.compile()
    return nc


def _get_nc():
    if "nc" not in _STATE:
        _STATE["nc"] = _build()
    return _STATE["nc"]


def _make_in_maps(x, mask, W_start, b_start, W_end, b_end, weight):
    cosT, sinT, sel, tril = _tables()
    x = np.asarray(x, np.float32)
    mask = np.ascontiguousarray(np.asarray(mask, np.float32))
    W_start = np.asarray(W_start, np.float32)
    W_end = np.asarray(W_end, np.float32)
    w_both = np.ascontiguousarray(np.concatenate([W_start, W_end], axis=1))
    bias2 = np.ascontiguousarray(
        np.concatenate([np.asarray(b_start, np.float32).reshape(H),
                        np.asarray(b_end, np.float32).reshape(H)]).reshape(
                            2 * H, 1))
    weight = np.ascontiguousarray(np.asarray(weight, np.float32))
    in_maps = []
    for c in range(NCORES):
        b, half = c // 2, c % 2
        in_maps.append({
            "xT": np.ascontiguousarray(x[b].T),
            "mask": np.ascontiguousarray(mask[b:b + 1]),
            "w_both": w_both,
            "bias2": bias2,
            "w_o": np.ascontiguousarray(weight[half * OH:(half + 1) * OH]),
            "cos_t": cosT,
            "sin_t": sinT,
            "sel3": sel,
            "trilneg": tril,
        })
    return in_maps


def _execute(in_maps, trace=False):
    from concourse.bass_utils import run_bass_kernel_spmd
    nc = _get_nc()
    return run_bass_kernel_spmd(nc, in_maps, list(range(NCORES)), trace=trace)


def kernel(x, mask, W_start, b_start, W_end, b_end, weight):
    in_maps = _make_in_maps(x, mask, W_start, b_start, W_end, b_end, weight)
    res = _execute(in_maps)
    outs = [res.results[c]["out"] for c in range(NCORES)]
    full = np.stack(outs).reshape(B, 2, OH, S, S).reshape(B, O, S, S)
    return full.astype(np.float32)
